# revision 51
# baseline (speedup 1.0000x reference)
"""Bahdanau attention fused kernel for Trainium2, 8-core data-parallel.

Reference computation (per batch b of 32, H=1024, S=2048):
    enc_score = encoder_out @ We + be                    [B, S, H]
    dec_score = dec @ Wd + bd                            [B, 1, H]
    score     = tanh(enc_score + dec_score)              [B, S, H]
    ls        = score @ Ws + bs                          [B, S, 1]
    w         = softmax(ls, axis=S)
    out       = sum_s w[b,s] * encoder_out[b,s,:]        [B, H]

Sharding: batch 32 -> 4 per core across 8 cores; weights replicated.
The tiny dec-score GEMM is folded into the host-side bias preparation:
bias[b] = be + bd + dec[b] @ Wd. bs is dropped (softmax shift-invariant).
No max-subtraction in softmax: |ls| <= 16.

fp8 version: the big X@We GEMM and the score@Ws projection run in
fp8e4m3 with perf_mode=DoubleRow (2 fp8 weights per PE cell -> one
matmul contracts 256 rows).  The context accumulation keeps a separate
bf16 copy of X and fuses multiply+reduce into single-pass
tensor_tensor_reduce ops on VectorE.

Per-core device layout (h-partitioned, prepared host-side):
    xt8  [4, 4, 128, 8, 512] fp8  xt8[b,c,p,k,s'] = X[b, c*512+s', k*128+p]
    xtb  [4, 4, 128, 8, 512] bf16 same values in bf16 (context path)
    we   [128, 8, 1024]      fp8  we[p,k,n]       = We[k*128+p, n]
    ws   [128, 8, 16]        fp8  ws[p,j,0]       = Ws[j*128+p, 0] (rest 0)
    bias [128, 32]           f32  bias[p, j*4+b]  = (be+bd+dec[b]@Wd)[j*128+p]
    out: ctx [4, 128, 8]     f32  ctx[b,p,j]      = out[b, j*128+p]

Device schedule per (b, c) chunk:
  - 8 j-tiles x 4 DoubleRow matmuls (k-pairs) -> PSUM [128, 512]
  - ScalarE evacuates with fused tanh(psum + bias[b,j]) -> fp8 score,
    written into [128, 2, 512] j-pair tiles
  - ls.T = 4 DoubleRow matmuls (ws j-pairs x score pairs) -> PSUM [1,512]
  - ScalarE exp (bf16) with fused accum_out denominator (fp32)
  - context: exp weights broadcast to 128 partitions on GpSimd, then per
    k-tile one fused tensor_tensor_reduce (VectorE) accumulates
    ctx partials; deferred one chunk so nothing blocks the PE stream.
"""

import numpy as np
import ml_dtypes

import concourse.tile as tile
from concourse import bacc, mybir
from concourse.bass_utils import run_bass_kernel_spmd

FP8 = mybir.dt.float8e4
BF16 = mybir.dt.bfloat16
F32 = mybir.dt.float32
AF = mybir.ActivationFunctionType
ALU = mybir.AluOpType
DR = mybir.MatmulPerfMode.DoubleRow

N_CORES = 8
H = 1024
S = 2048
B_PER_CORE = 4
S_CHUNK = 512

# We/Ws are uniform(-1/32, 1/32) — below e4m3's min normal 2^-6 they
# quantize to subnormals (3.5x the noise).  Scale them up by 64 before
# the fp8 cast and fold 1/64 into the ScalarE activation scale (free).
W_SCALE = 64.0

# Feature flags (HW bring-up bisection)
MAIN_DR = True   # fp8 DoubleRow for the X@We GEMM
LS_DR = True     # fp8 DoubleRow for the score@Ws projection
USE_AMR = True   # fused affine_mul_reduce for the context path

# test.py can flip this to get a profiled run; the grading path never does.
PROFILE = {"trace": False, "tmpdir": None}


def build_program(b_per_core=B_PER_CORE, s=S, h=H):
    kt = h // 128
    jt = h // 128
    n_sc = s // S_CHUNK
    nc = bacc.Bacc("TRN2", target_bir_lowering=False, debug=False)

    xt8_d = nc.dram_tensor(
        "xt8", [b_per_core, n_sc, 128, kt, S_CHUNK], FP8, kind="ExternalInput"
    ).ap()
    xtb_d = nc.dram_tensor(
        "xtb", [b_per_core, n_sc, 128, kt, S_CHUNK], BF16, kind="ExternalInput"
    ).ap()
    we_d = nc.dram_tensor("we", [128, kt, h], FP8, kind="ExternalInput").ap()
    ws_d = nc.dram_tensor("ws", [128, jt, 16], FP8, kind="ExternalInput").ap()
    bias_d = nc.dram_tensor(
        "bias", [128, jt * b_per_core], F32, kind="ExternalInput"
    ).ap()
    ctx_d = nc.dram_tensor("ctx", [b_per_core, 128, jt], F32, kind="ExternalOutput").ap()
    # tail chunk of the last batch in [s, h] layout: its context runs on the
    # (otherwise idle) PE at the very end instead of the serial DVE chain
    xst_d = nc.dram_tensor(
        "xst", [S_CHUNK // 128, 128, h], BF16, kind="ExternalInput"
    ).ap()
    ctxt_d = nc.dram_tensor("ctxt", [1, h], F32, kind="ExternalOutput").ap()

    with tile.TileContext(nc) as tc:
        with (
            tc.tile_pool(name="consts", bufs=1) as consts,
            tc.tile_pool(name="xt8p", bufs=8) as xt8p,
            tc.tile_pool(name="xtbp", bufs=5) as xtbp,
            tc.tile_pool(name="scorep", bufs=10) as scorep,
            tc.tile_pool(name="smallp", bufs=2 * n_sc) as smallp,
            tc.tile_pool(name="ebcp", bufs=2 * n_sc) as ebcp,
            tc.tile_pool(name="trashp", bufs=2) as trashp,
            tc.tile_pool(name="ctxp", bufs=2) as ctxp,
            # 6 main banks let the PE run two j-groups ahead of the tanh
            # evacuations; ls gets the other 2 (the tail broadcast borrows
            # a buf from ls since it runs after the last ls group).
            tc.tile_pool(name="ps_main", bufs=6, space="PSUM") as ps_main,
            tc.tile_pool(name="ps_ls", bufs=1, space="PSUM") as ps_ls,
            tc.tile_pool(name="ps_tail", bufs=1, space="PSUM") as ps_tail,
        ):
            # Head DMAs: a dma_start costs ~0.6us of issue time on its
            # queue, so the gate-opening transfers are spread across the
            # four idle engine queues to fire in parallel (sync gets the
            # first xt8 k-pair below, scalar gets we k-pair 0).
            we_sb = consts.tile([128, kt, h], FP8)
            we_queues = [nc.scalar, nc.scalar, nc.scalar, nc.scalar]
            for kp in range(kt // 2):
                we_queues[kp].dma_start(
                    we_sb[:, 2 * kp : 2 * kp + 2, :], we_d[:, 2 * kp : 2 * kp + 2, :]
                )
            ws_sb = consts.tile([128, jt, 16], FP8)
            nc.scalar.dma_start(ws_sb[:], ws_d[:])
            bias_sb = consts.tile([128, jt * b_per_core], F32)
            nc.scalar.dma_start(bias_sb[:], bias_d[:])
            eye1 = consts.tile([1, 1], BF16)
            nc.vector.memset(eye1[:], 1.0)
            xst_tiles = []

            def emit_context_chunk(xtb_bc, ex, ctx4_b, c):
                """Broadcast chunk weights and accumulate context partials."""
                ebc = ebcp.tile([128, S_CHUNK], BF16, tag="ebc")
                nc.gpsimd.partition_broadcast(ebc[:], ex[:])
                for k in range(kt):
                    if USE_AMR:
                        # fused (xtb * ebc) multiply + free-axis reduce in one
                        # DVE pass (custom-ucode op; the ISA-level
                        # TENSOR_TENSOR_REDUCE doesn't execute on this runtime)
                        trash = trashp.tile([128, S_CHUNK], BF16, tag="trash")
                        nc.vector.affine_mul_reduce(
                            trash[:],
                            ctx4_b[:, k * n_sc + c : k * n_sc + c + 1],
                            xtb_bc[:, k, :],
                            ebc[:],
                            scale=1.0,
                            bias=0.0,
                        )
                    else:
                        scr = trashp.tile([128, S_CHUNK], BF16, tag="trash")
                        nc.vector.tensor_mul(scr[:], xtb_bc[:, k, :], ebc[:])
                        nc.vector.reduce_sum(
                            ctx4_b[:, k * n_sc + c : k * n_sc + c + 1],
                            scr[:],
                            axis=mybir.AxisListType.X,
                        )

            def emit_invd(denom_b, want_bc=True):
                """softmax denominator -> 1/d as [1,1] (+ [128,1] spread).

                The partition spread runs on GpSimd: a PE ones-matmul here
                would sit in the PE FIFO waiting on the exp chain and stall
                the next batch's main matmul stream (~2us per batch + a HAM
                re-throttle).
                """
                dsum = smallp.tile([1, 1], F32, tag="dsum")
                nc.vector.reduce_sum(dsum[:], denom_b[:], axis=mybir.AxisListType.X)
                invd = smallp.tile([1, 1], F32, tag="invd")
                nc.vector.reciprocal(invd[:], dsum[:])
                invd_bc = smallp.tile([128, 1], F32, tag="invdbc")
                nc.gpsimd.partition_broadcast(invd_bc[:], invd[:])
                return invd, invd_bc

            def emit_tail_half_amr(xtb_bc, ex, tailcol):
                """Second s-half (256:512) of the tail chunk via the AMR
                path, into its own [128, kt] column tile (so the batch-final
                reduces don't have to wait for it).  Runs on GpSimd+DVE
                concurrently with the PE half."""
                hs = S_CHUNK // 2
                ebc = ebcp.tile([128, hs], BF16, tag="ebct")
                nc.gpsimd.partition_broadcast(ebc[:], ex[:, hs:])
                for k in range(kt):
                    trash = trashp.tile([128, hs], BF16, tag="trasht")
                    nc.vector.affine_mul_reduce(
                        trash[:],
                        tailcol[:, k : k + 1],
                        xtb_bc[:, k, hs:],
                        ebc[:],
                        scale=1.0,
                        bias=0.0,
                    )

            def emit_tail_ctx(ex, invd):
                """First s-half (0:256) of the tail chunk on the PE.

                The exp weights are spread across partitions with two
                transpose-mode matmuls, evacuated on the (idle) ScalarE, then
                contracted against the [s, h]-layout copy of the chunk.  The
                result is normalized into its own [1, h] output that the
                host adds to the last batch's row.
                """
                nt = S_CHUNK // 2 // 128
                wt_ps = ps_tail.tile([128, S_CHUNK], BF16, tag="tailbc")
                for i in range(nt):
                    nc.tensor.matmul(
                        wt_ps[:, i * 128 : i * 128 + 1],
                        lhsT=ex[:, i * 128 : (i + 1) * 128],
                        rhs=eye1[:],
                        is_transpose=True,
                        start=(i == 0),
                        stop=(i == nt - 1),
                    )
                wt_sb = smallp.tile([128, nt], BF16, tag="wt")
                for i in range(nt):
                    # ScalarE, not DVE: the DVE is busy with the AMR chains
                    nc.scalar.copy(
                        wt_sb[:, i : i + 1], wt_ps[:, i * 128 : i * 128 + 1]
                    )
                ctxt_sb = ctxp.tile([1, h], F32, tag="ctxt")
                for half in range(h // S_CHUNK):
                    # half 1 takes the (now free) tail bank: going through
                    # the single ls bank would serialize these matmuls
                    # behind half 0's evacuation on the busy DVE
                    pool = ps_ls if half == 0 else ps_tail
                    tag = "ls" if half == 0 else "tailbc"
                    ct_ps = pool.tile([1, S_CHUNK], F32, tag=tag)
                    for i in range(nt):
                        nc.tensor.matmul(
                            ct_ps[:],
                            lhsT=wt_sb[:, i : i + 1],
                            rhs=xst_tiles[i][:, half * S_CHUNK : (half + 1) * S_CHUNK],
                            start=(i == 0),
                            stop=(i == nt - 1),
                        )
                    # normalize on ScalarE — the DVE is busy with the AMRs
                    nc.scalar.activation(
                        ctxt_sb[:, half * S_CHUNK : (half + 1) * S_CHUNK],
                        ct_ps[:],
                        AF.Identity,
                        scale=invd[:],
                    )
                nc.sync.dma_start(ctxt_d[:], ctxt_sb[:])

            def emit_batch_reduce(ctx4_b, n_c=None):
                n_c = n_sc if n_c is None else n_c
                ctxu = ctxp.tile([128, jt], F32, tag="ctxu")
                for k in range(kt):
                    nc.vector.reduce_sum(
                        ctxu[:, k : k + 1],
                        ctx4_b[:, k * n_sc : k * n_sc + n_c],
                        axis=mybir.AxisListType.X,
                    )
                return ctxu

            def emit_batch_final(b, ctx4_b, invd_bc, ctxu=None, tailcol=None):
                """Partial reduction, normalize, store."""
                if ctxu is None:
                    ctxu = emit_batch_reduce(ctx4_b)
                if tailcol is not None:
                    ctxs = ctxp.tile([128, jt], F32, tag="ctxs")
                    nc.vector.tensor_add(ctxs[:], ctxu[:], tailcol[:])
                    ctxu = ctxs
                ctx_b = ctxp.tile([128, jt], F32, tag="ctx")
                nc.vector.tensor_scalar_mul(ctx_b[:], ctxu[:], invd_bc[:])
                nc.sync.dma_start(ctx_d[b], ctx_b[:])

            pending = []  # deferred (context-chunk | invd | batch-final)
            for b in range(b_per_core):
                xt8_tiles = []
                xtb_tiles = []
                for c in range(n_sc):
                    xt8_bc = xt8p.tile([128, kt, S_CHUNK], FP8, tag="xt8")
                    if b == 0 and c == 0:
                        # split the gate-opening chunk by k-pair so the
                        # first matmul group starts on the first slice
                        for kp in range(kt // 2):
                            nc.sync.dma_start(
                                xt8_bc[:, 2 * kp : 2 * kp + 2, :],
                                xt8_d[b, c][:, 2 * kp : 2 * kp + 2, :],
                            )
                    elif b == 0 and c % 2 == 1:
                        # batch 0 is the DMA-gated stretch: odd chunks ride
                        # the scalar queue (idle after the we slices) so the
                        # sync queue's issue slots all feed the PE gate
                        nc.scalar.dma_start(xt8_bc[:], xt8_d[b, c])
                    else:
                        nc.sync.dma_start(xt8_bc[:], xt8_d[b, c])
                    xt8_tiles.append(xt8_bc)
                for c in range(n_sc):
                    # xtb rides the sync queue too: a dma_start costs ~0.6us
                    # on its issuing engine queue, and on ScalarE that issue
                    # cost delayed tanh evacuations enough to stall the PE
                    # at every batch boundary.  Issued after the batch's xt8
                    # so the fp8 stream (which gates the PE) goes first.
                    xtb_bc = xtbp.tile([128, kt, S_CHUNK], BF16, tag="xtb")
                    nc.sync.dma_start(xtb_bc[:], xtb_d[b, c])
                    xtb_tiles.append(xtb_bc)

                if b == b_per_core - 1:
                    # [s, h]-layout copy of the tail chunk's first half for
                    # the PE-side context; issued late so it never competes
                    # with the head's gating stream
                    for i in range(S_CHUNK // 2 // 128):
                        xst_t = consts.tile([128, h], BF16, tag=f"xst{i}")
                        nc.scalar.dma_start(xst_t[:], xst_d[i])
                        xst_tiles.append(xst_t)

                denom_b = smallp.tile([1, n_sc], F32, tag="denom")
                ctx4_b = ctxp.tile([128, kt * n_sc], F32, tag="ctx4")
                for c in range(n_sc):
                    ls_ps = ps_ls.tile([1, S_CHUNK], F32, tag="ls")
                    score_pairs = []
                    for j in range(jt):
                        jp, jh = divmod(j, 2)
                        if jh == 0:
                            sc = scorep.tile([128, 2, S_CHUNK], FP8, tag="score")
                            score_pairs.append(sc)
                        mm_ps = ps_main.tile([128, S_CHUNK], F32, tag="main")
                        if MAIN_DR:
                            for kp in range(kt // 2):
                                nc.tensor.matmul(
                                    mm_ps[:],
                                    lhsT=we_sb[:, 2 * kp : 2 * kp + 2, j * 128 : (j + 1) * 128],
                                    rhs=xt8_tiles[c][:, 2 * kp : 2 * kp + 2, :],
                                    start=(kp == 0),
                                    stop=(kp == kt // 2 - 1),
                                    perf_mode=DR,
                                )
                        else:
                            for k in range(kt):
                                nc.tensor.matmul(
                                    mm_ps[:],
                                    lhsT=we_sb[:, k, j * 128 : (j + 1) * 128],
                                    rhs=xt8_tiles[c][:, k, :],
                                    start=(k == 0),
                                    stop=(k == kt - 1),
                                )
                        nc.scalar.activation(
                            score_pairs[jp][:, jh, :], mm_ps[:], AF.Tanh,
                            scale=1.0 / W_SCALE,
                            bias=bias_sb[:, j * b_per_core + b : j * b_per_core + b + 1],
                        )
                        if j == 0:
                            # deferred work from the previous chunk/batch is
                            # emitted right after the first matmul group, so
                            # its PE ops slot in early and the DVE context
                            # work overlaps this chunk's remaining groups
                            for fn in pending:
                                fn()
                            pending = []
                        if LS_DR and j >= 3 and j % 2 == 1 and j < jt - 1:
                            # interleave the ls accumulation into the main
                            # stream two groups behind the tanh chain, so
                            # only the final j-pair's ls matmul sits on the
                            # chunk-end critical path
                            jp = (j - 3) // 2
                            nc.tensor.matmul(
                                ls_ps[:],
                                lhsT=ws_sb[:, 2 * jp : 2 * jp + 2, 0:1],
                                rhs=score_pairs[jp][:],
                                start=(jp == 0),
                                stop=False,
                                perf_mode=DR,
                            )
                    if LS_DR:
                        for jp in range(jt // 2 - 2, jt // 2):
                            nc.tensor.matmul(
                                ls_ps[:],
                                lhsT=ws_sb[:, 2 * jp : 2 * jp + 2, 0:1],
                                rhs=score_pairs[jp][:],
                                start=False,
                                stop=(jp == jt // 2 - 1),
                                perf_mode=DR,
                            )
                    else:
                        for j in range(jt):
                            nc.tensor.matmul(
                                ls_ps[:],
                                lhsT=ws_sb[:, j, 0:1],
                                rhs=score_pairs[j // 2][:, j % 2, :],
                                start=(j == 0),
                                stop=(j == jt - 1),
                            )
                    last_b = b == b_per_core - 1
                    tail = last_b and c == n_sc - 1

                    def emit_exp(ls_ps=ls_ps, denom_b=denom_b, c=c, tail=tail):
                        # the exp is deferred off the chunk boundary so it
                        # lands AFTER the next chunk's first tanh in the
                        # ScalarE FIFO — otherwise it adds ~0.7us of latency
                        # to every chunk's tanh->ls critical chain
                        ex = smallp.tile([1, S_CHUNK], BF16, tag="exp")
                        nc.scalar.activation(
                            ex[:], ls_ps[:], AF.Exp, scale=1.0 / W_SCALE,
                            accum_out=denom_b[:, c : c + 1]
                        )
                        return ex

                    if tail:
                        # tail of the whole kernel: the last chunk's context
                        # is split — second s-half on GpSimd+DVE, first
                        # s-half on the PE — so the two chains drain in
                        # parallel
                        ex = emit_exp()
                        invd, invd_bc = emit_invd(denom_b)
                        # chunks 0-2 reduce first: they're long done, and
                        # this keeps the DVE's end-of-kernel chain to just
                        # the AMRs + a [128,8] add + scale
                        ctxu = emit_batch_reduce(ctx4_b, n_c=n_sc - 1)
                        tailcol = ctxp.tile([128, kt], F32, tag="tailcol")
                        emit_tail_half_amr(xtb_tiles[c], ex, tailcol)
                        emit_tail_ctx(ex, invd)
                        emit_batch_final(b, ctx4_b, invd_bc, ctxu=ctxu,
                                         tailcol=tailcol)
                    elif last_b:
                        # no deferral on the last batch: there are no later
                        # main matmuls to hide behind, so get the softmax ->
                        # context chains started as early as possible
                        ex = emit_exp()
                        pending.append(
                            lambda xtb_bc=xtb_tiles[c], ex=ex, ctx4_b=ctx4_b,
                            c=c: emit_context_chunk(xtb_bc, ex, ctx4_b, c)
                        )
                    elif c < n_sc - 1:
                        def chunk_tail(emit_exp=emit_exp, xtb_bc=xtb_tiles[c],
                                       ctx4_b=ctx4_b, c=c):
                            ex = emit_exp()
                            emit_context_chunk(xtb_bc, ex, ctx4_b, c)
                        pending.append(chunk_tail)
                    else:
                        def batch_tail(emit_exp=emit_exp, xtb_bc=xtb_tiles[c],
                                       b=b, ctx4_b=ctx4_b, c=c,
                                       denom_b=denom_b):
                            ex = emit_exp()
                            _, invd_bc = emit_invd(denom_b)
                            emit_context_chunk(xtb_bc, ex, ctx4_b, c)
                            emit_batch_final(b, ctx4_b, invd_bc)
                        pending.append(batch_tail)

    nc.compile()
    return nc


_CACHED = {}


def _get_program(key):
    if key not in _CACHED:
        _CACHED[key] = build_program(*key)
    return _CACHED[key]


def make_in_maps(encoder_out, decoder_hidden_state, We, be, Wd, bd, Ws, bs,
                 b_per_core=B_PER_CORE, s=S, h=H, n_cores=N_CORES):
    kt = h // 128
    jt = h // 128
    n_sc = s // S_CHUNK
    bf = ml_dtypes.bfloat16
    f8 = ml_dtypes.float8_e4m3

    we_a = np.ascontiguousarray(
        (We * W_SCALE).reshape(kt, 128, h).transpose(1, 0, 2)
    ).astype(f8)
    ws_a = np.zeros((128, jt, 16), dtype=np.float32)
    ws_a[:, :, 0] = (Ws[:, 0] * W_SCALE).reshape(jt, 128).T
    ws_a = ws_a.astype(f8)

    dec = decoder_hidden_state[0]  # [32, h]
    bias_all = (be + bd)[None, :] + dec @ Wd  # [32, h] fp32
    in_maps = []
    for i in range(n_cores):
        b0 = i * b_per_core
        xb = encoder_out[b0 : b0 + b_per_core]  # [b, s, h]
        # [b, c, s', k, p] -> [b, c, p, k, s']
        xt5 = np.ascontiguousarray(
            xb.reshape(b_per_core, n_sc, S_CHUNK, kt, 128).transpose(0, 1, 4, 3, 2)
        )
        bias_a = np.ascontiguousarray(
            bias_all[b0 : b0 + b_per_core].reshape(b_per_core, jt, 128).transpose(2, 1, 0)
        ).reshape(128, jt * b_per_core).astype(np.float32)
        # tail chunk (last batch, last chunk) in [s-tile, s-part, h] layout
        xst_a = np.ascontiguousarray(
            xb[b_per_core - 1, (n_sc - 1) * S_CHUNK :].reshape(
                S_CHUNK // 128, 128, h
            )
        ).astype(bf)
        in_maps.append({
            "xt8": xt5.astype(f8),
            "xtb": xt5.astype(bf),
            "xst": xst_a,
            "we": we_a,
            "ws": ws_a,
            "bias": bias_a,
        })
    return in_maps


def kernel(encoder_out, decoder_hidden_state, We, be, Wd, bd, Ws, bs):
    encoder_out = np.asarray(encoder_out, dtype=np.float32)
    decoder_hidden_state = np.asarray(decoder_hidden_state, dtype=np.float32)
    We = np.asarray(We, dtype=np.float32)
    be = np.asarray(be, dtype=np.float32)
    Wd = np.asarray(Wd, dtype=np.float32)
    bd = np.asarray(bd, dtype=np.float32)
    Ws = np.asarray(Ws, dtype=np.float32)
    bs = np.asarray(bs, dtype=np.float32)

    nc = _get_program((B_PER_CORE, S, H))
    in_maps = make_in_maps(
        encoder_out, decoder_hidden_state, We, be, Wd, bd, Ws, bs
    )
    kwargs = {}
    if PROFILE["trace"]:
        kwargs = {"trace": True, "tmpdir": PROFILE["tmpdir"]}
    res = run_bass_kernel_spmd(nc, in_maps, list(range(N_CORES)), **kwargs)
    PROFILE["last_result"] = res

    out = np.empty((N_CORES * B_PER_CORE, H), dtype=np.float32)
    for i in range(N_CORES):
        ctx = res.results[i]["ctx"]  # [b, 128, jt]
        out[i * B_PER_CORE : (i + 1) * B_PER_CORE] = (
            ctx.transpose(0, 2, 1).reshape(B_PER_CORE, H)
        )
        # last batch's tail-chunk context came back on the PE path
        out[(i + 1) * B_PER_CORE - 1] += res.results[i]["ctxt"][0]
    return out
